# revision 22
# baseline (speedup 1.0000x reference)
"""Trainium2 Bass/Tile kernel for nn_CAVAModule (cross-attention A/V alignment).

Math notes (exact simplifications of the reference):
  - delta = 2 + 4*sigmoid(clip(theta,-12,12)) is in [2, 6], so the mask
    center min(max(t+delta,0),t) == t for every t: the displacement-aware
    causal mask is a fixed 6-tap causal moving average, independent of theta.
  - The soft temporal shift composed with that moving average is a banded
    Toeplitz operator: three 128x128 blocks C00/Csub/Cdiag applied as PE
    matmuls against 128-token LayerNorm'd audio tiles.
  - l2_normalize(LN(x)) collapses: vn = w_v * rstd_v / sqrt(DM) exactly,
    so the video branch needs no separate sum of squares.
  - rsqrt via the magic-constant Newton iteration on the DVE, batched per
    4-tile group - no Sqrt ACT-table switches (only gelu_and_others is
    ever loaded). The 1-iteration Newton for 1/||a_ctx|| returns the
    NEGATIVE root; the sign is folded into W1's an and an*vn blocks.
  - The +-12 logit clip is dropped: sigmoid(+-12) lies far outside the
    [0.05, 0.95] gate clip, so the gate clip subsumes it.
  - Gate MLP in fp8e4 DoubleRow (2 k-tiles per matmul); W1 host-scaled
    (x32/x32/x2 => logits x512, gelu applies 1/512) and stored k-pair-
    interleaved so the moving operand streams contiguous byte pairs.

Structure: 4-tile groups; phase A (projections+stats) of group g is
interleaved tile-by-tile with phase C (gate MLP + fuse) of group g-1 so
the PE always has projection matmuls to fill gate-path dependency gaps.

Dtypes: video/audio/Wv/Wa bf16, context matmul f32r, gate path fp8/bf16,
output staged bf16 (host converts to f32). End-to-end rel err ~3e-3
(budget 2e-2).

Sharding: data-parallel over batch, one sample per NeuronCore, no
cross-core communication. All DRAM operands are host-relaid contiguous
per partition (128-descriptor DMAs); output is partition-major,
unpermuted on the host.
"""

import sys

for _p in ("/opt/trn_rl_repo",):
    if _p not in sys.path:
        sys.path.insert(0, _p)

import ml_dtypes
import numpy as np

import concourse.bacc as bacc
import concourse.bass as bass
import concourse.tile as tile
from concourse import mybir
from concourse.bass_utils import run_bass_kernel_spmd

F32 = mybir.dt.float32
F32R = mybir.dt.float32r
BF16 = mybir.dt.bfloat16
F8 = mybir.dt.float8e4
U32 = mybir.dt.uint32
ALU = mybir.AluOpType
ACT = mybir.ActivationFunctionType
DR = mybir.MatmulPerfMode.DoubleRow

B, T, VDIM, ADIM, DM, HID = 8, 2048, 1024, 768, 256, 1024
P = 128
NT = T // P          # 16 token tiles
KV = VDIM // P       # 8
KA = ADIM // P       # 6
KX = (3 * DM) // P   # 6
NB = 8               # token tiles per stats batch / input DMA chunk
NG = NT // NB        # 4 groups
LN_EPS = 1e-5
WIN = 6              # mask window taps (tau in [t-5, t])
CGATE = 512.0        # gate-logit scale carried through the fp8 MLP
MAGIC = 0x5F3759DF   # rsqrt Newton seed

_nc_cache: dict = {}


def _build_cmats(delta: float) -> np.ndarray:
    """Three [tau, t] blocks of the combined shift+mask operator."""
    dl = min(max(delta, 0.0), float(T - 1))
    n = int(np.floor(dl))
    alpha = dl - n

    def row_w(t):
        w = np.zeros(2 * P, np.float64)
        m = min(t + 1, WIN)
        for s in range(max(0, t - (WIN - 1)), t + 1):
            i0 = min(max(s - n, 0), T - 1)
            i1 = min(i0 + 1, T - 1)
            w[i0] += (1.0 - alpha) / m
            w[i1] += alpha / m
        return w

    c00 = np.zeros((P, P), np.float64)
    csub = np.zeros((P, P), np.float64)
    cdiag = np.zeros((P, P), np.float64)
    for t in range(P):
        w = row_w(t)
        c00[:, t] = w[:P]
        w = row_w(P + t)
        csub[:, t] = w[:P]
        cdiag[:, t] = w[P:2 * P]
    return np.ascontiguousarray(np.stack([c00, csub, cdiag]).astype(np.float32))


def _build(bv_nz: bool, ba_nz: bool, b1_nz: bool, b2f: float):
    from contextlib import ExitStack

    nc = bacc.Bacc("TRN2", target_bir_lowering=False, debug=False, num_devices=8)

    vtc = nc.dram_tensor("vtc", [P, NT, KV, P], BF16, kind="ExternalInput")
    atc = nc.dram_tensor("atc", [P, NT, KA, P], BF16, kind="ExternalInput")
    wv = nc.dram_tensor("wv", [P, KV, DM], BF16, kind="ExternalInput")
    wa = nc.dram_tensor("wa", [P, KA, DM], BF16, kind="ExternalInput")
    w1 = nc.dram_tensor("w1", [P, KX // 2, HID, 2], F8, kind="ExternalInput")
    w2h = nc.dram_tensor("w2h", [HID], BF16, kind="ExternalInput")
    cm = nc.dram_tensor("cm", [P, 3, P], F32R, kind="ExternalInput")
    ident = nc.dram_tensor("ident", [P, P], BF16, kind="ExternalInput")
    if bv_nz:
        bvr = nc.dram_tensor("bvr", [1, DM], F32R, kind="ExternalInput")
    if ba_nz:
        bar = nc.dram_tensor("bar", [1, DM], F32R, kind="ExternalInput")
    if b1_nz:
        b1r = nc.dram_tensor("b1r", [1, HID], F32R, kind="ExternalInput")
    out = nc.dram_tensor("out", [P, NT, DM], BF16, kind="ExternalOutput")

    def bcast(handle_ap, n):
        return bass.AP(
            tensor=handle_ap.tensor, offset=handle_ap.offset, ap=[[0, P], [1, n]]
        )

    with tile.TileContext(nc) as tc:
        with ExitStack() as stk:
            singles = stk.enter_context(tc.tile_pool(name="singles", bufs=1))
            vchunk = stk.enter_context(tc.tile_pool(name="vchunk", bufs=2))
            achunk = stk.enter_context(tc.tile_pool(name="achunk", bufs=2))
            wv_pool = stk.enter_context(tc.tile_pool(name="wvp", bufs=2 * NB + 2))
            wan_pool = stk.enter_context(tc.tile_pool(name="wan", bufs=NB + 1))
            cms_pool = stk.enter_context(tc.tile_pool(name="cms", bufs=2))
            cx_pool = stk.enter_context(tc.tile_pool(name="cxp", bufs=NB + 2))
            nv_pool = stk.enter_context(tc.tile_pool(name="nvp", bufs=3))
            xt_pool = stk.enter_context(tc.tile_pool(name="xtp", bufs=3))
            hbuf = stk.enter_context(tc.tile_pool(name="hb", bufs=3))
            obuf = stk.enter_context(tc.tile_pool(name="ob", bufs=2))
            small = stk.enter_context(tc.tile_pool(name="small", bufs=10))
            bsm = stk.enter_context(tc.tile_pool(name="bsm", bufs=2))
            psum_mm = stk.enter_context(
                tc.tile_pool(name="psum_mm", bufs=2, space="PSUM"))
            psum_pc = stk.enter_context(
                tc.tile_pool(name="psum_pc", bufs=2, space="PSUM"))
            psum_tr = stk.enter_context(
                tc.tile_pool(name="psum_tr", bufs=2, space="PSUM"))
            psum_h = stk.enter_context(
                tc.tile_pool(name="psum_h", bufs=2, space="PSUM"))
            # ---- persistent weights/constants ----
            # sync queue first: wv gates the very first matmul
            wv_sb = singles.tile([P, KV, DM], BF16)
            nc.sync.dma_start(out=wv_sb, in_=wv.ap())
            # gpsimd queue: everything else, in need order
            wa_sb = singles.tile([P, KA, DM], BF16)
            nc.gpsimd.dma_start(out=wa_sb, in_=wa.ap())
            cm_sb = singles.tile([P, 3, P], F32R)
            nc.gpsimd.dma_start(out=cm_sb, in_=cm.ap())
            id_sb = singles.tile([P, P], BF16)
            nc.gpsimd.dma_start(out=id_sb, in_=ident.ap())
            w1_sb = singles.tile([P, KX // 2, HID, 2], F8)
            nc.gpsimd.dma_start(out=w1_sb, in_=w1.ap())
            w2_sb = singles.tile([P, HID], BF16)
            nc.gpsimd.dma_start(out=w2_sb, in_=bcast(w2h.ap(), HID))
            nrstd_all = singles.tile([P, 2, NT], F32)
            kmag = singles.tile([P, 2, NB], U32)
            nc.vector.memset(kmag, MAGIC)
            if bv_nz or ba_nz or b1_nz:
                ones_sb = singles.tile([1, P], F32R)
                nc.vector.memset(ones_sb, 1.0)
            if bv_nz:
                bv_sb = singles.tile([1, DM], F32R)
                nc.gpsimd.dma_start(out=bv_sb, in_=bvr.ap())
            if ba_nz:
                ba_sb = singles.tile([1, DM], F32R)
                nc.gpsimd.dma_start(out=ba_sb, in_=bar.ap())
            if b1_nz:
                b1_sb = singles.tile([1, HID], F32R)
                nc.gpsimd.dma_start(out=b1_sb, in_=b1r.ap())

            a_prev = [None]

            def phase_a_tile(g, j, vt_sb, at_sb, S):
                """Projections + stats + centering for tile (g, j)."""
                pva = psum_mm.tile([P, 2, DM], F32, tag="mm")
                pv = pva[:, 0, :]
                pa = pva[:, 1, :]
                for k in range(KV):
                    nc.tensor.matmul(pv, lhsT=vt_sb[:, j, k, :],
                                     rhs=wv_sb[:, k, :],
                                     start=(k == 0),
                                     stop=(k == KV - 1 and not bv_nz))
                if bv_nz:
                    nc.tensor.matmul(pv, lhsT=ones_sb, rhs=bv_sb,
                                     start=False, stop=True)
                stats = small.tile([P, nc.vector.BN_STATS_DIM], F32, tag="st",
                                   name=f"sv{g}_{j}")
                nc.vector.bn_stats(out=stats, in_=pv)
                nc.vector.bn_aggr(out=S["mv"][:, 0, j, :], in_=stats)
                w_v = wv_pool.tile([P, DM], BF16, tag="wv")
                nc.vector.tensor_scalar(out=w_v, in0=pv,
                                        scalar1=S["mv"][:, 0, j, 0:1],
                                        scalar2=None, op0=ALU.subtract)
                S["w_v"][j] = w_v

                for k in range(KA):
                    nc.tensor.matmul(pa, lhsT=at_sb[:, j, k, :],
                                     rhs=wa_sb[:, k, :],
                                     start=(k == 0),
                                     stop=(k == KA - 1 and not ba_nz))
                if ba_nz:
                    nc.tensor.matmul(pa, lhsT=ones_sb, rhs=ba_sb,
                                     start=False, stop=True)
                stats2 = small.tile([P, nc.vector.BN_STATS_DIM], F32, tag="st",
                                    name=f"sa{g}_{j}")
                nc.vector.bn_stats(out=stats2, in_=pa)
                nc.vector.bn_aggr(out=S["mv"][:, 1, j, :], in_=stats2)
                # w_a_neg = mean_a - pa  (ACT Identity; negation undone by
                # the negated LN scale in phase B)
                w_an = wan_pool.tile([P, DM], BF16, tag="wan")
                nc.scalar.activation(out=w_an, in_=pa, func=ACT.Identity,
                                     bias=S["mv"][:, 1, j, 0:1], scale=-1.0)
                S["w_an"][j] = w_an

            def newton_rstd(g, S):
                """rstd = 1/sqrt(var+eps) for v & a rows, 2 iterations."""
                nx = bsm.tile([P, 2, NB], F32, tag="nx")
                nc.vector.tensor_scalar(out=nx[:, :, :, None],
                                        in0=S["mv"][:, :, :, 1:2],
                                        scalar1=LN_EPS, scalar2=None, op0=ALU.add)
                nxh = bsm.tile([P, 2, NB], F32, tag="nxh")
                nc.vector.tensor_scalar(out=nxh, in0=nx, scalar1=0.5,
                                        scalar2=None, op0=ALU.mult)
                nt0 = bsm.tile([P, 2, NB], F32, tag="nt0")
                nt1 = bsm.tile([P, 2, NB], F32, tag="nt1")
                yb = bsm.tile([P, 2, NB], F32, tag="yb")
                yc = bsm.tile([P, 2, NB], F32, tag="yc")
                nc.vector.tensor_scalar(out=nt0.bitcast(U32), in0=nx.bitcast(U32),
                                        scalar1=1, scalar2=None,
                                        op0=ALU.logical_shift_right)
                nc.vector.tensor_tensor(out=yb.bitcast(U32), in0=kmag,
                                        in1=nt0.bitcast(U32), op=ALU.subtract)
                # one Newton step yields the NEGATIVE root directly: -rstd
                nc.vector.tensor_tensor(out=nt0, in0=yb, in1=yb, op=ALU.mult)
                nc.vector.tensor_tensor(out=nt1, in0=nt0, in1=nxh, op=ALU.mult)
                nrs = nrstd_all[:, :, g * NB:(g + 1) * NB]
                nc.vector.scalar_tensor_tensor(out=nrs, in0=nt1,
                                               scalar=1.5, in1=yb,
                                               op0=ALU.subtract, op1=ALU.mult)
                nc.vector.tensor_scalar(out=S["rstd"], in0=nrs,
                                        scalar1=-1.0, scalar2=None, op0=ALU.mult)
                # fold the (negated) audio LN scale into the mask matrices:
                # cms_tap[:, j, :] = cm_tap * (-rstd_a[token-tile]) so the ctx
                # matmul can consume w_an = (mean - pa) directly
                cms_d = cms_pool.tile([P, NB, P], BF16, tag="cmd", name="cms_d")
                cms_s = cms_pool.tile([P, NB, P], BF16, tag="cmsu", name="cms_s")
                def b_cm(tap):
                    c = cm_sb[:, tap, :]
                    return bass.AP(tensor=c.tensor, offset=c.offset,
                                   ap=[c.ap[0], [0, NB], c.ap[1]])
                def b_nr(lo):
                    n = nrstd_all[:, 1, lo:lo + NB]
                    return bass.AP(tensor=n.tensor, offset=n.offset,
                                   ap=[n.ap[0], n.ap[1], [0, P]])
                nc.vector.tensor_tensor(out=cms_d, in0=b_cm(2),
                                        in1=b_nr(g * NB), op=ALU.mult)
                if g == 0:
                    nc.vector.tensor_tensor(out=cms_s[:, 0:1, :],
                                            in0=b_cm(0)[:, 0:1, :],
                                            in1=b_nr(0)[:, 0:1, :],
                                            op=ALU.mult)
                    nc.vector.tensor_tensor(out=cms_s[:, 1:NB, :],
                                            in0=b_cm(1)[:, 0:NB - 1, :],
                                            in1=b_nr(0)[:, 0:NB - 1, :],
                                            op=ALU.mult)
                else:
                    nc.vector.tensor_tensor(out=cms_s, in0=b_cm(1),
                                            in1=b_nr(g * NB - 1), op=ALU.mult)
                S["cms_d"] = cms_d
                S["cms_s"] = cms_s

            def phase_b_tile(g, j, S):
                """Context matmul on w_an with LN scale folded into cms."""
                pc = psum_pc.tile([P, DM], F32, tag="pc")
                if g == 0 and j == 0:
                    nc.tensor.matmul(pc, lhsT=S["cms_s"][:, 0, :],
                                     rhs=S["w_an"][0], start=True, stop=True)
                else:
                    nc.tensor.matmul(pc, lhsT=S["cms_s"][:, j, :],
                                     rhs=a_prev[0], start=True, stop=False)
                    nc.tensor.matmul(pc, lhsT=S["cms_d"][:, j, :],
                                     rhs=S["w_an"][j], start=False, stop=True)
                a_prev[0] = S["w_an"][j]
                actx = cx_pool.tile([P, DM], BF16, tag="cx")
                nc.scalar.activation(out=actx, in_=pc, func=ACT.Copy)
                S["actx"][j] = actx
                sqd = nv_pool.tile([P, DM], BF16, tag="sq")
                nc.vector.scalar_tensor_tensor(
                    out=sqd, in0=actx, scalar=1.0 / 256.0, in1=actx,
                    op0=ALU.mult, op1=ALU.mult,
                    accum_out=S["ssq"][:, j:j + 1])

            def newton_rn16(g, S, lo=0, hi=NB):
                """rn16 = -16/||a_ctx|| (negative root, sign folded into W1),
                computed for tiles [lo, hi) of the group."""
                w = hi - lo
                sq = S["ssq"][:, lo:hi]
                n2h = bsm.tile([P, NB], F32, tag="n2h", name="n2h", bufs=4)[:, 0:w]
                nc.vector.tensor_scalar(out=n2h, in0=sq, scalar1=0.5,
                                        scalar2=None, op0=ALU.mult)
                n2t = bsm.tile([P, NB], F32, tag="n2t", name="n2t", bufs=4)[:, 0:w]
                n2y = bsm.tile([P, NB], F32, tag="n2y", name="n2y", bufs=4)[:, 0:w]
                n2u = bsm.tile([P, NB], F32, tag="n2u", name="n2u", bufs=4)[:, 0:w]
                nc.vector.tensor_scalar(out=n2t.bitcast(U32),
                                        in0=sq.bitcast(U32),
                                        scalar1=1, scalar2=None,
                                        op0=ALU.logical_shift_right)
                nc.vector.tensor_tensor(out=n2y.bitcast(U32),
                                        in0=kmag[:, 0, 0:w],
                                        in1=n2t.bitcast(U32), op=ALU.subtract)
                nc.vector.tensor_tensor(out=n2t, in0=n2y, in1=n2y, op=ALU.mult)
                nc.vector.tensor_tensor(out=n2u, in0=n2t, in1=n2h, op=ALU.mult)
                nc.vector.scalar_tensor_tensor(out=S["rn16"][:, lo:hi], in0=n2u,
                                               scalar=1.5, in1=n2y,
                                               op0=ALU.subtract, op1=ALU.mult)

            def phase_c_tile(g, j, S):
                """Gate MLP + fuse for tile (g, j)."""
                i_glob = g * NB + j
                w_v = S["w_v"][j]
                actx = S["actx"][j]
                an_s = nv_pool.tile([P, DM], BF16, tag="an")
                nc.scalar.activation(out=an_s, in_=actx, func=ACT.Copy,
                                     scale=S["rn16"][:, j:j + 1])
                vn_s = nv_pool.tile([P, DM], BF16, tag="vn")
                nc.scalar.activation(out=vn_s, in_=w_v, func=ACT.Copy,
                                     scale=S["rstd"][:, 0, j:j + 1])
                avn = nv_pool.tile([P, DM], BF16, tag="avn")
                nc.vector.tensor_tensor(out=avn, in0=an_s, in1=vn_s,
                                        op=ALU.mult)
                pt = psum_tr.tile([P, KX, P], BF16, tag="pt")
                for k in range(2):
                    nc.tensor.transpose(pt[:, k, :],
                                        an_s[:, k * P:(k + 1) * P], id_sb)
                for k in range(2):
                    nc.tensor.transpose(pt[:, 2 + k, :],
                                        vn_s[:, k * P:(k + 1) * P], id_sb)
                for k in range(2):
                    nc.tensor.transpose(pt[:, 4 + k, :],
                                        avn[:, k * P:(k + 1) * P], id_sb)
                xt = xt_pool.tile([P, KX, P], F8, tag="xt")
                nc.scalar.activation(out=xt, in_=pt, func=ACT.Copy)

                ph0 = psum_h.tile([P, 512], F32, tag="h")
                ph1 = psum_h.tile([P, 512], F32, tag="h")
                hh = hbuf.tile([P, HID], BF16, tag="hh")
                for nh, psl in ((0, ph0), (1, ph1)):
                    for kk in range(3):
                        nc.tensor.matmul(
                            psl, lhsT=xt[:, 2 * kk:2 * kk + 2, :],
                            rhs=w1_sb[:, kk, nh * 512:(nh + 1) * 512, :]
                            .rearrange("p n i -> p i n"),
                            start=(kk == 0), stop=(kk == 2 and not b1_nz),
                            perf_mode=DR)
                    if b1_nz:
                        nc.tensor.matmul(psl, lhsT=ones_sb,
                                         rhs=b1_sb[:, nh * 512:(nh + 1) * 512],
                                         start=False, stop=True)
                    nc.scalar.activation(out=hh[:, nh * 512:(nh + 1) * 512],
                                         in_=psl, func=ACT.Gelu,
                                         scale=1.0 / CGATE)

                lscd = hbuf.tile([P, HID], BF16, tag="lsc")
                lg = small.tile([P, 1], F32, tag="lg")
                nc.vector.scalar_tensor_tensor(out=lscd, in0=hh, scalar=0.0,
                                               in1=w2_sb, op0=ALU.bypass,
                                               op1=ALU.mult, accum_out=lg)
                # +-12 logit clip dropped: subsumed by the [0.05,0.95] clip
                if b2f != 0.0:
                    nc.vector.tensor_scalar(out=lg, in0=lg, scalar1=float(b2f),
                                            scalar2=None, op0=ALU.add)
                gg = small.tile([P, 1], F32, tag="gg")
                nc.scalar.activation(out=gg, in_=lg, func=ACT.Tanh, scale=0.5)
                nc.vector.tensor_scalar(out=gg, in0=gg, scalar1=0.5,
                                        scalar2=0.5, op0=ALU.mult, op1=ALU.add)
                nc.vector.tensor_scalar(out=gg, in0=gg, scalar1=0.05,
                                        scalar2=0.95, op0=ALU.max, op1=ALU.min)

                # fused = g*a_ctx + (1-g)*vln = vln + g*(a_ctx - vln)
                # where vln = w_v*rstd_v = vn_s exactly
                dd = nv_pool.tile([P, DM], BF16, tag="dd")
                nc.vector.tensor_tensor(out=dd, in0=actx, in1=vn_s,
                                        op=ALU.subtract)
                if j % 4 == 0:
                    S["ob"] = obuf.tile([P, 4, DM], BF16, tag="ob", name="ob")
                nc.vector.scalar_tensor_tensor(out=S["ob"][:, j % 4, :],
                                               in0=dd, scalar=gg, in1=vn_s,
                                               op0=ALU.mult, op1=ALU.add)
                if j % 4 == 3:
                    nc.sync.dma_start(
                        out=out.ap()[:, i_glob - 3:i_glob + 1, :], in_=S["ob"])

            def new_state(g):
                return {
                    "mv": bsm.tile([P, 2, NB, 2], F32, tag="mv", name="mv"),
                    "rstd": bsm.tile([P, 2, NB], F32, tag="rstd", name="rstd"),
                    "nrstd": bsm.tile([P, 2, NB], F32, tag="nrs", name="nrstd"),
                    "ssq": bsm.tile([P, NB], F32, tag="ssq", name="ssq"),
                    "rn16": bsm.tile([P, NB], F32, tag="rn16", name="rn16"),
                    "w_v": [None] * NB, "w_an": [None] * NB,
                    "actx": [None] * NB, "ob": None,
                }

            Sprev = None
            for g in range(NG):
                i0 = g * NB
                vt_sb = vchunk.tile([P, NB, KV, P], BF16, tag="vt")
                at_sb = achunk.tile([P, NB, KA, P], BF16, tag="at")
                if g == 0:
                    for j in range(4):
                        nc.sync.dma_start(
                            out=vt_sb[:, j, :, :], in_=vtc.ap()[:, j, :, :])
                        nc.sync.dma_start(
                            out=at_sb[:, j, :, :], in_=atc.ap()[:, j, :, :])
                    if NB > 4:
                        nc.sync.dma_start(
                            out=vt_sb[:, 4:NB, :, :], in_=vtc.ap()[:, 4:NB, :, :])
                        nc.sync.dma_start(
                            out=at_sb[:, 4:NB, :, :], in_=atc.ap()[:, 4:NB, :, :])
                else:
                    nc.sync.dma_start(
                        out=vt_sb, in_=vtc.ap()[:, i0:i0 + NB, :, :])
                    nc.sync.dma_start(
                        out=at_sb, in_=atc.ap()[:, i0:i0 + NB, :, :])

                S = new_state(g)
                # phase A of group g interleaved with phase C of group g-1:
                # the PE alternates projection matmuls (always ready) with
                # the gate-path transposes/MLP whose inputs arrive late.
                for j in range(NB):
                    phase_a_tile(g, j, vt_sb, at_sb, S)
                    if Sprev is not None:
                        phase_c_tile(g - 1, j, Sprev)
                newton_rstd(g, S)
                if g < NG - 1:
                    for j in range(NB):
                        phase_b_tile(g, j, S)
                    newton_rn16(g, S)
                else:
                    # last group: rn16 in half-batches so phase C starts
                    # while the second half of phase B still runs
                    h = NB // 2
                    for j in range(h):
                        phase_b_tile(g, j, S)
                    newton_rn16(g, S, 0, h)
                    for j in range(h, NB):
                        phase_b_tile(g, j, S)
                        phase_c_tile(g, j - h, S)
                    newton_rn16(g, S, h, NB)
                    for j in range(h, NB):
                        phase_c_tile(g, j, S)
                Sprev = S

    nc.compile()
    return nc


def _prepare_in_maps(video_seq, audio_seq, Wv, bv, Wa, ba, theta, W1, b1, W2, b2):
    bf16 = ml_dtypes.bfloat16
    f8 = ml_dtypes.float8_e4m3
    video_seq = np.asarray(video_seq, np.float32)
    audio_seq = np.asarray(audio_seq, np.float32)
    th = float(np.clip(np.float32(theta), -12.0, 12.0))
    delta = 2.0 + 4.0 / (1.0 + np.exp(-th))
    cmats = _build_cmats(float(delta))

    bv_nz = bool(np.any(np.asarray(bv) != 0))
    ba_nz = bool(np.any(np.asarray(ba) != 0))
    b1_nz = bool(np.any(np.asarray(b1) != 0))
    b2f = float(np.asarray(b2).reshape(-1)[0])

    W1f = np.asarray(W1, np.float32)
    W1s = np.empty_like(W1f)
    W1s[:DM] = W1f[:DM] * (-CGATE / 16.0)          # sign-folds the negative rn16
    W1s[DM:2 * DM] = W1f[DM:2 * DM] * (CGATE / 16.0)
    W1s[2 * DM:] = W1f[2 * DM:] * (-CGATE / 256.0)
    # k-pair interleaved: [P, KX//2, HID, 2]
    w1r = np.ascontiguousarray(
        W1s.astype(f8).reshape(KX // 2, 2, P, HID).transpose(2, 0, 3, 1))

    def relay(w, ko):
        n = w.shape[1]
        return np.ascontiguousarray(w.reshape(ko, P, n).transpose(1, 0, 2))

    shared = {
        "wv": relay(np.asarray(Wv, np.float32).astype(bf16), KV),
        "wa": relay(np.asarray(Wa, np.float32).astype(bf16), KA),
        "w1": w1r,
        "w2h": np.ascontiguousarray(
            np.asarray(W2, np.float32).reshape(HID).astype(bf16)),
        "cm": np.ascontiguousarray(cmats.transpose(1, 0, 2)),
        "ident": np.eye(P, dtype=np.float32).astype(bf16),
    }
    if bv_nz:
        shared["bvr"] = np.ascontiguousarray(np.asarray(bv, np.float32).reshape(1, DM))
    if ba_nz:
        shared["bar"] = np.ascontiguousarray(np.asarray(ba, np.float32).reshape(1, DM))
    if b1_nz:
        shared["b1r"] = np.ascontiguousarray(
            np.asarray(b1, np.float32).reshape(1, HID) * CGATE)

    in_maps = []
    for b in range(B):
        m = dict(shared)
        m["vtc"] = np.ascontiguousarray(
            video_seq[b].T.astype(bf16).reshape(KV, P, NT, P).transpose(1, 2, 0, 3))
        m["atc"] = np.ascontiguousarray(
            audio_seq[b].T.astype(bf16).reshape(KA, P, NT, P).transpose(1, 2, 0, 3))
        in_maps.append(m)
    return in_maps, (bv_nz, ba_nz, b1_nz, b2f)


def kernel(video_seq, audio_seq, Wv, bv, Wa, ba, theta, W1, b1, W2, b2):
    in_maps, key = _prepare_in_maps(video_seq, audio_seq, Wv, bv, Wa, ba,
                                    theta, W1, b1, W2, b2)
    if key not in _nc_cache:
        _nc_cache[key] = _build(*key)
    nc = _nc_cache[key]
    res = run_bass_kernel_spmd(nc, in_maps, list(range(B)))
    outs = []
    for i in range(B):
        r = np.asarray(res.results[i]["out"]).astype(np.float32)
        outs.append(np.ascontiguousarray(
            r.reshape(P, NT, DM).transpose(1, 0, 2).reshape(T, DM)))
    return np.stack(outs)
